# revision 1
# baseline (speedup 1.0000x reference)
"""Trainium2 Bass kernel for nn_CustomCIFAR10Model.

Math (reference):
    xf = x.reshape(B, D)
    part2[b,d] = cos(xf[b,d]) * Sa[d] + sin(xf[b,d]) * Sb[d]
        where Sa[d] = sum_i a[i,d,0], Sb[d] = sum_i b[i,d,0]
    part1 = sum(w[1:]*n[1:] + w[:-1]*n[:-1])            (scalar)
    out = (part1 + part2) @ fc_w.T + fc_b               [B, NCLS]

The heavy part is reading a and b (2 x 37.75 MB) once to column-sum them:
memory-bound. Sharding: columns (d) split across 8 cores, 384 each. Every
core independently column-sums its a/b slice (PE matmuls against a ones
vector, f32r moving operand = full-rate single-pass), builds z = cos*Sa +
sin*Sb for its d-slice, and contracts against its fc_w columns, yielding
a partial [NCLS, B] output. Host sums the 8 partials and adds part1/bias
(part1 contributes part1 * rowsum(fc_w) to every batch row).

Schedule: trig on x (ACT+DVE) is issued first so it overlaps the long
a/b DMA stream; after the last accumulation only the Sa/Sb-dependent
muls, one add, and three f32r matmuls remain as the tail.

HW Sin only accepts [-pi, pi]: range-reduce t = x/(2pi), r = t - round(t)
via the fp32 magic-number trick, then Sin(2pi*r); cos shifts t by +1/4.
"""

import numpy as np

B = 512
D = 3072
NCLS = 100
P = 128
NCORES = 8
DW = D // NCORES          # 384 columns per core
NSUB = DW // P            # 3 d-subtiles of 128
NCH = D // P              # 24 row-chunks of a/b slice

_STATE = {}


def _build():
    """Build + bacc-compile the SPMD Bass program (once per process)."""
    import concourse.bacc as bacc
    import concourse.mybir as mybir
    import concourse.tile as tile

    f32 = mybir.dt.float32
    f32r = mybir.dt.float32r
    nc = bacc.Bacc(
        "TRN2", target_bir_lowering=False, debug=False, num_devices=NCORES
    )

    # a/b/fwt declared f32r: same fp32 bits; the PE truncates the mantissa
    # during the (full-rate, single-pass) f32r matmul whether or not the
    # inputs were pre-rounded, so no cast op is needed. The host packs
    # a/b as [NGRP, P, GRP*DW]: partition-major groups so every DMA run
    # is GRP*DW*4 = 6KB contiguous.
    GRP = 4                    # row-chunks per packed group
    NGRP = NCH // GRP          # 6 packed groups per tensor
    a_s = nc.dram_tensor("a_s", [NGRP, P, GRP * DW], f32r, kind="ExternalInput")
    b_s = nc.dram_tensor("b_s", [NGRP, P, GRP * DW], f32r, kind="ExternalInput")
    xt_s = nc.dram_tensor("xt_s", [DW, B], f32, kind="ExternalInput")
    fwt_s = nc.dram_tensor("fwt_s", [DW, NCLS], f32r, kind="ExternalInput")
    out_cb = nc.dram_tensor("out_cb", [NCLS, B], f32, kind="ExternalOutput")

    INV2PI = float(1.0 / (2.0 * np.pi))
    TWO_PI = float(2.0 * np.pi)
    MAGIC = float(1.5 * 2.0**23)
    add_op = mybir.AluOpType.add
    sub_op = mybir.AluOpType.subtract
    mult_op = mybir.AluOpType.mult
    Sin = mybir.ActivationFunctionType.Sin
    Copy = mybir.ActivationFunctionType.Copy

    with tile.TileContext(nc) as tc:
        with (
            tc.tile_pool(name="chunks", bufs=6) as chunk_pool,
            tc.tile_pool(name="consts", bufs=1) as const_pool,
            tc.tile_pool(name="xwork", bufs=1) as x_pool,
            tc.tile_pool(name="ps", bufs=2, space="PSUM") as psum_pool,
            tc.tile_pool(name="psrow", bufs=1, space="PSUM") as psum_row_pool,
            tc.tile_pool(name="psout", bufs=1, space="PSUM") as psum_out_pool,
        ):
            ones_f = const_pool.tile([P, 1], f32, name="ones_f")
            nc.vector.memset(ones_f[:], 1.0)
            ones = const_pool.tile([P, 1], f32r, name="ones")
            nc.vector.tensor_copy(ones[:], ones_f[:])
            zero = const_pool.tile([P, 1], f32, name="zerob")
            nc.vector.memset(zero[:], 0.0)
            # Dummy Sin so the Sin table set loads once at kernel start;
            # Copy is a filler in every set, so later Copy ACTIVATEs on
            # the scalar engine reuse the resident set (no reload).
            warm = const_pool.tile([P, 1], f32, name="warm")
            nc.scalar.activation(warm[:], zero[:], Sin, bias=zero[:])

            srcs = (a_s, b_s)
            rows = []
            for ti in range(2):
                psr = psum_row_pool.tile([1, DW], f32, name=f"psr{ti}", tag=f"psr{ti}")
                rows.append(psr)

            emitted = [0, 0]

            def load_chunks(ti, g, j0, nj):
                """One DMA for chunks [GRP*g+j0, +nj) + their matmuls.
                PSUM accumulation is order-insensitive, so start/stop
                follow emission order, not chunk index."""
                ch = chunk_pool.tile(
                    [P, nj, DW], f32r, name=f"ch{ti}_{g}_{j0}", tag="chunk"
                )
                src_ap = srcs[ti][g, :, j0 * DW : (j0 + nj) * DW]
                nc.sync.dma_start(out=ch[:], in_=src_ap)
                for j in range(nj):
                    nc.tensor.matmul(
                        rows[ti][:],
                        ones[:],
                        ch[:, j, :],
                        start=(emitted[ti] == 0),
                        stop=(emitted[ti] == NCH - 1),
                    )
                    emitted[ti] += 1

            # xt/fwt issue from the scalar queue so they don't delay the
            # a-stream on sync; trig overlaps the stream either way.
            xt = x_pool.tile([P, NSUB, B], f32, name="xt")
            nc.scalar.dma_start(
                out=xt[:], in_=xt_s[:].rearrange("(s p) b -> p s b", p=P)
            )
            fwt = x_pool.tile([P, NSUB, NCLS], f32r, name="fwt")
            nc.scalar.dma_start(
                out=fwt[:], in_=fwt_s[:].rearrange("(s p) c -> p s c", p=P)
            )
            for g in range(NGRP):
                load_chunks(0, g, 0, GRP)

            # Trig on x while a/b stream: r = t - round(t) (magic trick),
            # then Sin(2pi*r); cos shifts t by +1/4 before rounding.
            sins = []
            coss = []
            for sub in range(NSUB):
                xts = xt[:, sub, :]
                ts_t = x_pool.tile([P, B], f32, name=f"ts{sub}", tag=f"ts{sub}")
                nc.scalar.activation(ts_t[:], xts, Copy, bias=0.0, scale=INV2PI)
                ks_t = x_pool.tile([P, B], f32, name=f"ks{sub}", tag=f"ks{sub}")
                nc.vector.tensor_scalar(ks_t[:], ts_t[:], MAGIC, MAGIC, add_op, sub_op)
                nc.vector.tensor_sub(ts_t[:], ts_t[:], ks_t[:])
                sinv = x_pool.tile([P, B], f32, name=f"sin{sub}", tag=f"sin{sub}")
                nc.scalar.activation(
                    sinv[:], ts_t[:], Sin, bias=zero[:], scale=TWO_PI
                )
                # pre-cast to f32r mid-stream so the tail matmul can use
                # it directly (Sa/Sb scaling moves to the tiny fwt tiles)
                sinr = x_pool.tile([P, B], f32r, name=f"sinr{sub}", tag=f"sinr{sub}")
                nc.vector.tensor_copy(sinr[:], sinv[:])
                sins.append(sinr)
                tc_t = x_pool.tile([P, B], f32, name=f"tc{sub}", tag=f"tc{sub}")
                nc.scalar.activation(tc_t[:], xts, Copy, bias=0.25, scale=INV2PI)
                kc_t = x_pool.tile([P, B], f32, name=f"kc{sub}", tag=f"kc{sub}")
                nc.vector.tensor_scalar(kc_t[:], tc_t[:], MAGIC, MAGIC, add_op, sub_op)
                nc.vector.tensor_sub(tc_t[:], tc_t[:], kc_t[:])
                cosv = x_pool.tile([P, B], f32, name=f"cos{sub}", tag=f"cos{sub}")
                nc.scalar.activation(
                    cosv[:], tc_t[:], Sin, bias=zero[:], scale=TWO_PI
                )
                cosr = x_pool.tile([P, B], f32r, name=f"cosr{sub}", tag=f"cosr{sub}")
                nc.vector.tensor_copy(cosr[:], cosv[:])
                coss.append(cosr)

            out_ps = psum_out_pool.tile([NCLS, B], f32, name="out_ps")

            def finish_tensor(ti, vals, col_base, start):
                """Transpose row ti to per-partition cols, scale the SMALL
                fwt tiles by them (fwt[d,c]*S[d]), and accumulate
                (fwt*S).T @ trig into out_ps — the wide trig tiles need no
                tail-side scaling."""
                for sub in range(NSUB):
                    # per-sub row tiles: transpose(sub0) fires after a
                    # ~200ns copy instead of one 544ns monolithic copy
                    row_sb = const_pool.tile(
                        [1, P], f32, name=f"row{ti}_{sub}", tag=f"row{ti}_{sub}"
                    )
                    nc.vector.tensor_copy(
                        row_sb[:], rows[ti][0:1, sub * P : (sub + 1) * P]
                    )
                    ps = psum_pool.tile([P, 1], f32, name=f"ps{ti}_{sub}", tag="ps")
                    nc.tensor.matmul(
                        ps[:],
                        row_sb[:],
                        ones_f[0:1, 0:1],
                        start=True,
                        stop=True,
                    )
                    fws = x_pool.tile(
                        [P, NCLS], f32r, name=f"fws{ti}_{sub}", tag=f"fws{ti}{sub}"
                    )
                    nc.vector.tensor_scalar_mul(fws[:], fwt[:, sub, :], ps[:])
                    nc.tensor.matmul(
                        out_ps[:],
                        fws[:],
                        vals[sub][:],
                        start=(start and sub == 0),
                        stop=(not start and sub == NSUB - 1),
                    )

            # a finishes mid-stream: its cos-side output matmuls overlap
            # the b stream. b's sin side forms the (short) tail; b's last
            # group is split so the final matmuls start ~0.6us earlier.
            finish_tensor(0, coss, 0, start=True)
            # The last group's chunks are issued FIRST: per-queue DMAs
            # transfer roughly FIFO, so the small final pieces are resident
            # long before the tail and never stall on recycled semaphores;
            # the last-arriving data is then a full mid group.
            load_chunks(1, NGRP - 1, 0, 2)
            load_chunks(1, NGRP - 1, 2, 2)
            for g in range(NGRP - 1):
                load_chunks(1, g, 0, GRP)
            finish_tensor(1, sins, 3, start=False)

            # Split store: two half-copies on different engines, two DMA
            # queues, so copy/DMA of the halves overlap.
            out_sb = const_pool.tile([NCLS, B], f32, name="out_sb")
            H = B // 2
            nc.scalar.copy(out_sb[:, 0:H], out_ps[:, 0:H])
            nc.scalar.dma_start(out=out_cb[:, 0:H], in_=out_sb[:, 0:H])
            nc.vector.tensor_copy(out_sb[:, H:B], out_ps[:, H:B])
            nc.sync.dma_start(out=out_cb[:, H:B], in_=out_sb[:, H:B])

    nc.compile()
    return nc


def _get_nc():
    if "nc" not in _STATE:
        _STATE["nc"] = _build()
    return _STATE["nc"]


def _pack(t2, sl):
    """[D, DW] slice -> [NGRP, P, GRP*DW]: 12KB-contiguous DMA runs."""
    ngrp, grp = NCH // 4, 4
    s = t2[:, sl].reshape(ngrp, grp, P, DW).transpose(0, 2, 1, 3)
    return np.ascontiguousarray(s).reshape(ngrp, P, grp * DW)


def _prep_in_maps(x, a, b, fc_w):
    xf = np.ascontiguousarray(np.asarray(x, dtype=np.float32).reshape(B, D))
    xt = np.ascontiguousarray(xf.T)  # [D, B]
    a2 = np.asarray(a, dtype=np.float32).reshape(D, D)
    b2 = np.asarray(b, dtype=np.float32).reshape(D, D)
    fw = np.asarray(fc_w, dtype=np.float32)
    in_maps = []
    for m in range(NCORES):
        sl = slice(m * DW, (m + 1) * DW)
        in_maps.append(
            {
                "a_s": _pack(a2, sl),
                "b_s": _pack(b2, sl),
                "xt_s": np.ascontiguousarray(xt[sl, :]),
                "fwt_s": np.ascontiguousarray(fw[:, sl].T),
            }
        )
    return in_maps


def _run(inputs, trace=False, trace_kwargs=None):
    """Run the device kernel; returns (final_output, BassKernelResults)."""
    from concourse.bass_utils import run_bass_kernel_spmd

    x = inputs["x"]
    a = inputs["a"]
    b = inputs["b"]
    w = np.asarray(inputs["w"], dtype=np.float64)
    n_param = np.asarray(inputs["n_param"], dtype=np.float64)
    fc_w = np.asarray(inputs["fc_w"], dtype=np.float32)
    fc_b = np.asarray(inputs["fc_b"], dtype=np.float32)

    nc = _get_nc()
    in_maps = _prep_in_maps(x, a, b, fc_w)
    res = run_bass_kernel_spmd(
        nc,
        in_maps,
        list(range(NCORES)),
        trace=trace,
        **(trace_kwargs or {}),
    )

    acc = np.zeros((NCLS, B), dtype=np.float32)
    for r in res.results:
        acc += r["out_cb"]
    part1 = float(np.sum(w[1:] * n_param[1:] + w[:-1] * n_param[:-1]))
    final = acc.T + np.float32(part1) * fc_w.sum(axis=1)[None, :] + fc_b[None, :]
    return np.ascontiguousarray(final.astype(np.float32)), res


def kernel(**inputs) -> np.ndarray:
    out, _ = _run(inputs, trace=False)
    return out



# revision 2
# speedup vs baseline: 1.0476x; 1.0476x over previous
"""Trainium2 Bass kernel for nn_CustomCIFAR10Model.

Math (reference):
    xf = x.reshape(B, D)
    part2[b,d] = cos(xf[b,d]) * Sa[d] + sin(xf[b,d]) * Sb[d]
        where Sa[d] = sum_i a[i,d,0], Sb[d] = sum_i b[i,d,0]
    part1 = sum(w[1:]*n[1:] + w[:-1]*n[:-1])            (scalar)
    out = (part1 + part2) @ fc_w.T + fc_b               [B, NCLS]

The heavy part is reading a and b once to column-sum them: memory-bound.
Sharding: columns (d) split across 8 cores, 384 each. Every core
column-sums its a/b slice (PE matmuls against a ones vector), builds
z = cos*Sa + sin*Sb for its d-slice, and contracts against its fc_w
columns, yielding a partial [NCLS, B] output. Host sums the 8 partials
and adds part1/bias (part1 contributes part1 * rowsum(fc_w) per class).

a/b/x are cast to bf16 on the host (tolerance is 2e-2; measured error
stays ~1e-3): halves the dominant HBM traffic vs f32 and doubles the
PE matmul rate (bf16 moving operand streams 1 col/cycle).

All input DMAs ride one HWDGE queue (sync) in FIFO order
fwt -> xt -> a0..a2 -> b0..b3: 8 DMAs fit the 8 HWDGE completion
semaphores exactly, so no issue ever stalls on semaphore recycling,
and the queue streams back-to-back at full HBM rate. Host packs every
tensor partition-major so each DMA is contiguous >=1.5KB runs per
partition. The last b group is only 2 chunks so almost no matmul work
remains after the final byte lands.

HW Sin only accepts [-pi, pi]: range-reduce t = x/(2pi), r = t - round(t)
via the fp32 magic-number trick, then Sin(2pi*r); cos shifts t by +1/4.
"""

import numpy as np

B = 512
D = 3072
NCLS = 100
P = 128
NCORES = 8
DW = D // NCORES          # 384 columns per core
NSUB = DW // P            # 3 d-subtiles of 128
NCH = D // P              # 24 row-chunks of a/b slice
GROUPS = [(0, 8), (8, 8), (16, 8)]          # a stream
GROUPS_B = [(0, 8), (8, 8), (16, 6), (22, 2)]  # b stream, small tail

_STATE = {}


def _build():
    """Build + bacc-compile the SPMD Bass program (once per process)."""
    import concourse.bacc as bacc
    import concourse.mybir as mybir
    import concourse.tile as tile

    f32 = mybir.dt.float32
    bf16 = mybir.dt.bfloat16
    nc = bacc.Bacc(
        "TRN2", target_bir_lowering=False, debug=False, num_devices=NCORES
    )

    a_s = nc.dram_tensor("a_s", [P, NCH * DW], bf16, kind="ExternalInput")
    b_s = nc.dram_tensor("b_s", [P, NCH * DW], bf16, kind="ExternalInput")
    xt_s = nc.dram_tensor("xt_s", [P, NSUB * B], bf16, kind="ExternalInput")
    fwt_s = nc.dram_tensor("fwt_s", [P, NSUB * NCLS], f32, kind="ExternalInput")
    out_cb = nc.dram_tensor("out_cb", [NCLS, B], f32, kind="ExternalOutput")

    INV2PI = float(1.0 / (2.0 * np.pi))
    TWO_PI = float(2.0 * np.pi)
    MAGIC = float(1.5 * 2.0**23)
    add_op = mybir.AluOpType.add
    sub_op = mybir.AluOpType.subtract
    Sin = mybir.ActivationFunctionType.Sin
    Copy = mybir.ActivationFunctionType.Copy

    with tile.TileContext(nc) as tc:
        with (
            tc.tile_pool(name="chunks", bufs=4) as chunk_pool,
            tc.tile_pool(name="consts", bufs=1) as const_pool,
            tc.tile_pool(name="xwork", bufs=1) as x_pool,
            tc.tile_pool(name="ps", bufs=2, space="PSUM") as psum_pool,
            tc.tile_pool(name="psrow", bufs=1, space="PSUM") as psum_row_pool,
            tc.tile_pool(name="psout", bufs=1, space="PSUM") as psum_out_pool,
        ):
            ones_bf = const_pool.tile([P, 1], bf16, name="ones_bf")
            nc.vector.memset(ones_bf[:], 1.0)
            ones_f = const_pool.tile([P, 1], f32, name="ones_f")
            nc.vector.memset(ones_f[:], 1.0)
            zero = const_pool.tile([P, 1], f32, name="zerob")
            nc.vector.memset(zero[:], 0.0)
            # Dummy Sin so the Sin table set loads once at kernel start;
            # Copy is a filler in every set, so later Copy ACTIVATEs on
            # the scalar engine reuse the resident set (no reload).
            warm = const_pool.tile([P, 1], f32, name="warm")
            nc.scalar.activation(warm[:], zero[:], Sin, bias=zero[:])

            # Input DMAs, all on the sync HWDGE queue, FIFO: the small
            # fwt/xt transfers land first (trig starts early), then the
            # a/b stream saturates HBM with zero issue stalls.
            fwt = x_pool.tile([P, NSUB, NCLS], f32, name="fwt")
            nc.sync.dma_start(out=fwt[:], in_=fwt_s[:])
            xt = x_pool.tile([P, NSUB, B], bf16, name="xt")
            nc.sync.dma_start(out=xt[:], in_=xt_s[:])

            srcs = (a_s, b_s)
            rows = []
            for ti in range(2):
                psr = psum_row_pool.tile([1, DW], f32, name=f"psr{ti}", tag=f"psr{ti}")
                rows.append(psr)
            emitted = [0, 0]

            def load_group(ti, c0, n):
                """One DMA for chunks [c0, c0+n) + their matmuls."""
                ch = chunk_pool.tile(
                    [P, n, DW], bf16, name=f"ch{ti}_{c0}", tag="chunk"
                )
                nc.sync.dma_start(
                    out=ch[:], in_=srcs[ti][:, c0 * DW : (c0 + n) * DW]
                )
                for j in range(n):
                    nc.tensor.matmul(
                        rows[ti][:],
                        ones_bf[:],
                        ch[:, j, :],
                        start=(emitted[ti] == 0),
                        stop=(emitted[ti] == NCH - 1),
                    )
                    emitted[ti] += 1

            for c0, n in GROUPS:
                load_group(0, c0, n)

            # Trig on x while a/b stream: r = t - round(t) (magic trick),
            # then Sin(2pi*r); cos shifts t by +1/4 before rounding.
            # Sin writes bf16 directly (matmul moving operand, no cast op).
            sins = []
            coss = []
            for sub in range(NSUB):
                xts = xt[:, sub, :]
                ts_t = x_pool.tile([P, B], f32, name=f"ts{sub}", tag=f"ts{sub}")
                nc.scalar.activation(ts_t[:], xts, Copy, bias=0.0, scale=INV2PI)
                ks_t = x_pool.tile([P, B], f32, name=f"ks{sub}", tag=f"ks{sub}")
                nc.vector.tensor_scalar(ks_t[:], ts_t[:], MAGIC, MAGIC, add_op, sub_op)
                nc.vector.tensor_sub(ts_t[:], ts_t[:], ks_t[:])
                sinv = x_pool.tile([P, B], bf16, name=f"sin{sub}", tag=f"sin{sub}")
                nc.scalar.activation(
                    sinv[:], ts_t[:], Sin, bias=zero[:], scale=TWO_PI
                )
                sins.append(sinv)
                tc_t = x_pool.tile([P, B], f32, name=f"tc{sub}", tag=f"tc{sub}")
                nc.scalar.activation(tc_t[:], xts, Copy, bias=0.25, scale=INV2PI)
                kc_t = x_pool.tile([P, B], f32, name=f"kc{sub}", tag=f"kc{sub}")
                nc.vector.tensor_scalar(kc_t[:], tc_t[:], MAGIC, MAGIC, add_op, sub_op)
                nc.vector.tensor_sub(tc_t[:], tc_t[:], kc_t[:])
                cosv = x_pool.tile([P, B], bf16, name=f"cos{sub}", tag=f"cos{sub}")
                nc.scalar.activation(
                    cosv[:], tc_t[:], Sin, bias=zero[:], scale=TWO_PI
                )
                coss.append(cosv)

            out_ps = psum_out_pool.tile([NCLS, B], f32, name="out_ps")

            def finish_tensor(ti, vals, start):
                """Transpose row ti to per-partition cols, scale the SMALL
                fwt tiles by them (fwt[d,c]*S[d]), and accumulate
                (fwt*S).T @ trig into out_ps — the wide trig tiles need no
                tail-side scaling."""
                for sub in range(NSUB):
                    row_sb = const_pool.tile(
                        [1, P], f32, name=f"row{ti}_{sub}", tag=f"row{ti}_{sub}"
                    )
                    nc.vector.tensor_copy(
                        row_sb[:], rows[ti][0:1, sub * P : (sub + 1) * P]
                    )
                    ps = psum_pool.tile([P, 1], f32, name=f"ps{ti}_{sub}", tag="ps")
                    nc.tensor.matmul(
                        ps[:],
                        row_sb[:],
                        ones_f[0:1, 0:1],
                        start=True,
                        stop=True,
                    )
                    fws = x_pool.tile(
                        [P, NCLS], bf16, name=f"fws{ti}_{sub}", tag=f"fws{ti}{sub}"
                    )
                    nc.vector.tensor_scalar_mul(fws[:], fwt[:, sub, :], ps[:])
                    nc.tensor.matmul(
                        out_ps[:],
                        fws[:],
                        vals[sub][:],
                        start=(start and sub == 0),
                        stop=(not start and sub == NSUB - 1),
                    )

            # a finishes mid-stream: its cos-side output matmuls overlap
            # the b stream (trig is ready ~9us in, well before rows_a).
            finish_tensor(0, coss, start=True)
            for c0, n in GROUPS_B:
                load_group(1, c0, n)
            finish_tensor(1, sins, start=False)

            # Split store: two half-copies on different engines, two DMA
            # queues, so copy/DMA of the halves overlap.
            out_sb = const_pool.tile([NCLS, B], f32, name="out_sb")
            H = B // 2
            nc.scalar.copy(out_sb[:, 0:H], out_ps[:, 0:H])
            nc.scalar.dma_start(out=out_cb[:, 0:H], in_=out_sb[:, 0:H])
            nc.vector.tensor_copy(out_sb[:, H:B], out_ps[:, H:B])
            nc.sync.dma_start(out=out_cb[:, H:B], in_=out_sb[:, H:B])

    nc.compile()
    return nc


def _get_nc():
    if "nc" not in _STATE:
        _STATE["nc"] = _build()
    return _STATE["nc"]


def _pack_pm(t2, sl, np_dt):
    """[D rows, core cols] slice -> partition-major [P, NCH*DW]."""
    s = t2[:, sl].reshape(NCH, P, DW).transpose(1, 0, 2)
    return np.ascontiguousarray(s.astype(np_dt, copy=False)).reshape(P, NCH * DW)


def _prep_in_maps(x, a, b, fc_w):
    import ml_dtypes

    bf16 = ml_dtypes.bfloat16
    xf = np.asarray(x, dtype=np.float32).reshape(B, D)
    xtb = np.ascontiguousarray(xf.T).astype(bf16)  # [D, B] bf16
    a2 = np.asarray(a, dtype=np.float32).reshape(D, D).astype(bf16)
    b2 = np.asarray(b, dtype=np.float32).reshape(D, D).astype(bf16)
    fw = np.asarray(fc_w, dtype=np.float32)
    in_maps = []
    for m in range(NCORES):
        sl = slice(m * DW, (m + 1) * DW)
        xs = xtb[sl, :].reshape(NSUB, P, B).transpose(1, 0, 2)
        fs = np.ascontiguousarray(fw[:, sl].T).reshape(NSUB, P, NCLS)
        in_maps.append(
            {
                "a_s": _pack_pm(a2, sl, bf16),
                "b_s": _pack_pm(b2, sl, bf16),
                "xt_s": np.ascontiguousarray(xs).reshape(P, NSUB * B),
                "fwt_s": np.ascontiguousarray(
                    fs.transpose(1, 0, 2)
                ).reshape(P, NSUB * NCLS),
            }
        )
    return in_maps


def _run(inputs, trace=False, trace_kwargs=None):
    """Run the device kernel; returns (final_output, BassKernelResults)."""
    from concourse.bass_utils import run_bass_kernel_spmd

    x = inputs["x"]
    a = inputs["a"]
    b = inputs["b"]
    w = np.asarray(inputs["w"], dtype=np.float64)
    n_param = np.asarray(inputs["n_param"], dtype=np.float64)
    fc_w = np.asarray(inputs["fc_w"], dtype=np.float32)
    fc_b = np.asarray(inputs["fc_b"], dtype=np.float32)

    nc = _get_nc()
    in_maps = _prep_in_maps(x, a, b, fc_w)
    res = run_bass_kernel_spmd(
        nc,
        in_maps,
        list(range(NCORES)),
        trace=trace,
        **(trace_kwargs or {}),
    )

    acc = np.zeros((NCLS, B), dtype=np.float32)
    for r in res.results:
        acc += r["out_cb"]
    part1 = float(np.sum(w[1:] * n_param[1:] + w[:-1] * n_param[:-1]))
    final = acc.T + np.float32(part1) * fc_w.sum(axis=1)[None, :] + fc_b[None, :]
    return np.ascontiguousarray(final.astype(np.float32)), res


def kernel(**inputs) -> np.ndarray:
    out, _ = _run(inputs, trace=False)
    return out


# revision 3
# speedup vs baseline: 1.2435x; 1.1870x over previous
"""Trainium2 Bass kernel for nn_CustomCIFAR10Model.

Math (reference):
    xf = x.reshape(B, D)
    part2[b,d] = cos(xf[b,d]) * Sa[d] + sin(xf[b,d]) * Sb[d]
        where Sa[d] = sum_i a[i,d,0], Sb[d] = sum_i b[i,d,0]
    part1 = sum(w[1:]*n[1:] + w[:-1]*n[:-1])            (scalar)
    out = (part1 + part2) @ fc_w.T + fc_b               [B, NCLS]

The heavy part is reading a and b once to column-sum them: memory-bound.
Sharding: columns (d) split across 8 cores, 384 each. Every core
column-sums its a/b slice (PE matmuls), builds z = cos*Sa + sin*Sb for
its d-slice, and contracts against its fc_w columns, yielding a partial
[NCLS, B] output. Host sums the 8 partials and adds part1/bias.

a/b/x are cast to bf16 on the host (tolerance is 2e-2; measured error
~3e-3): halves the dominant HBM traffic vs f32 and doubles the PE
matmul rate. The column-sum matmuls use an all-ones [128,128] bf16
stationary so the [128, DW] output lands on all 128 PSUM partitions
(identical rows): a [1, DW] output would serialize on a single PSUM
write port (~1 value/ns), capping the stream at 382ns/chunk instead of
~160ns. Sa is then moved onto partitions with a one-hot e0 matmul
(stationary = SBUF copy of the sum rows, moving = e0 picks row 0).

Each group DMA reads one fully contiguous DRAM block (its own dram
tensor, packed partition-major on the host): a strided source costs
~20% of per-engine DMA bandwidth. All input DMAs ride one HWDGE queue
(sync) FIFO: fwt -> xt -> a0..a2 -> b0..b3 (8 DMAs = the 8 HWDGE
completion semaphores, so nothing ever stalls on semaphore recycling).
The last b group is only 2 chunks so almost no matmul work remains
after the final byte lands.

HW Sin only accepts [-pi, pi]: range-reduce t = x/(2pi), r = t - round(t)
via the fp32 magic-number trick, then Sin(2pi*r); cos shifts t by +1/4.
"""

import numpy as np

B = 512
D = 3072
NCLS = 100
P = 128
NCORES = 8
DW = D // NCORES          # 384 columns per core
NSUB = DW // P            # 3 d-subtiles of 128
NCH = D // P              # 24 row-chunks of a/b slice
GROUPS_A = [8, 8, 8]
GROUPS_B = [8, 8, 6, 2]   # small tail group

_STATE = {}


def _build():
    """Build + bacc-compile the SPMD Bass program (once per process)."""
    import concourse.bacc as bacc
    import concourse.mybir as mybir
    import concourse.tile as tile

    f32 = mybir.dt.float32
    bf16 = mybir.dt.bfloat16
    nc = bacc.Bacc(
        "TRN2", target_bir_lowering=False, debug=False, num_devices=NCORES
    )

    grp_srcs = [[], []]
    for ti, sizes in enumerate((GROUPS_A, GROUPS_B)):
        for gi, n in enumerate(sizes):
            grp_srcs[ti].append(
                nc.dram_tensor(
                    f"{'ab'[ti]}{gi}", [P, n * DW], bf16, kind="ExternalInput"
                )
            )
    xt_s = nc.dram_tensor("xt_s", [P, NSUB * B], bf16, kind="ExternalInput")
    fwt_s = nc.dram_tensor("fwt_s", [P, NSUB * NCLS], f32, kind="ExternalInput")
    out_cb = nc.dram_tensor("out_cb", [NCLS, B], f32, kind="ExternalOutput")

    INV2PI = float(1.0 / (2.0 * np.pi))
    TWO_PI = float(2.0 * np.pi)
    MAGIC = float(1.5 * 2.0**23)
    add_op = mybir.AluOpType.add
    sub_op = mybir.AluOpType.subtract
    Sin = mybir.ActivationFunctionType.Sin
    Copy = mybir.ActivationFunctionType.Copy

    with tile.TileContext(nc) as tc:
        with (
            tc.tile_pool(name="chunks", bufs=5) as chunk_pool,
            tc.tile_pool(name="consts", bufs=1) as const_pool,
            tc.tile_pool(name="xwork", bufs=1) as x_pool,
            tc.tile_pool(name="ps", bufs=2, space="PSUM") as psum_pool,
            tc.tile_pool(name="psrow", bufs=1, space="PSUM") as psum_row_pool,
            tc.tile_pool(name="psout", bufs=1, space="PSUM") as psum_out_pool,
        ):
            ones128 = const_pool.tile([P, P], bf16, name="ones128")
            nc.vector.memset(ones128[:], 1.0)
            e0 = const_pool.tile([P, 1], f32, name="e0")
            nc.vector.memset(e0[:], 0.0)
            nc.vector.memset(e0[0:1, 0:1], 1.0)
            zero = const_pool.tile([P, 1], f32, name="zerob")
            nc.vector.memset(zero[:], 0.0)
            # Dummy Sin so the Sin table set loads once at kernel start;
            # Copy is a filler in every set, so later Copy ACTIVATEs on
            # the scalar engine reuse the resident set (no reload).
            warm = const_pool.tile([P, 1], f32, name="warm")
            nc.scalar.activation(warm[:], zero[:], Sin, bias=zero[:])

            # Input DMAs, all on the sync HWDGE queue, FIFO: the small
            # fwt/xt transfers land first (trig starts early), then the
            # a/b stream saturates HBM with zero issue stalls.
            fwt = x_pool.tile([P, NSUB, NCLS], f32, name="fwt")
            nc.sync.dma_start(out=fwt[:], in_=fwt_s[:])
            xt = x_pool.tile([P, NSUB, B], bf16, name="xt")
            nc.sync.dma_start(out=xt[:], in_=xt_s[:])

            rows = []
            for ti in range(2):
                psr = psum_row_pool.tile(
                    [P, DW], f32, name=f"psr{ti}", tag=f"psr{ti}"
                )
                rows.append(psr)
            emitted = [0, 0]

            def load_group(ti, gi, n):
                """One DMA for one contiguous group + its matmuls."""
                ch = chunk_pool.tile(
                    [P, n, DW], bf16, name=f"ch{ti}_{gi}", tag="chunk"
                )
                nc.sync.dma_start(out=ch[:], in_=grp_srcs[ti][gi][:])
                for j in range(n):
                    nc.tensor.matmul(
                        rows[ti][:],
                        ones128[:],
                        ch[:, j, :],
                        start=(emitted[ti] == 0),
                        stop=(emitted[ti] == NCH - 1),
                    )
                    emitted[ti] += 1

            for gi, n in enumerate(GROUPS_A):
                load_group(0, gi, n)

            # Trig on x while a/b stream: r = t - round(t) (magic trick),
            # then Sin(2pi*r); cos shifts t by +1/4 before rounding.
            # Sin writes bf16 directly (matmul moving operand, no cast op).
            sins = []
            coss = []
            for sub in range(NSUB):
                xts = xt[:, sub, :]
                ts_t = x_pool.tile([P, B], f32, name=f"ts{sub}", tag=f"ts{sub}")
                nc.scalar.activation(ts_t[:], xts, Copy, bias=0.0, scale=INV2PI)
                ks_t = x_pool.tile([P, B], f32, name=f"ks{sub}", tag=f"ks{sub}")
                nc.vector.tensor_scalar(ks_t[:], ts_t[:], MAGIC, MAGIC, add_op, sub_op)
                nc.vector.tensor_sub(ts_t[:], ts_t[:], ks_t[:])
                sinv = x_pool.tile([P, B], bf16, name=f"sin{sub}", tag=f"sin{sub}")
                nc.scalar.activation(
                    sinv[:], ts_t[:], Sin, bias=zero[:], scale=TWO_PI
                )
                sins.append(sinv)
                tc_t = x_pool.tile([P, B], f32, name=f"tc{sub}", tag=f"tc{sub}")
                nc.scalar.activation(tc_t[:], xts, Copy, bias=0.25, scale=INV2PI)
                kc_t = x_pool.tile([P, B], f32, name=f"kc{sub}", tag=f"kc{sub}")
                nc.vector.tensor_scalar(kc_t[:], tc_t[:], MAGIC, MAGIC, add_op, sub_op)
                nc.vector.tensor_sub(tc_t[:], tc_t[:], kc_t[:])
                cosv = x_pool.tile([P, B], bf16, name=f"cos{sub}", tag=f"cos{sub}")
                nc.scalar.activation(
                    cosv[:], tc_t[:], Sin, bias=zero[:], scale=TWO_PI
                )
                coss.append(cosv)

            out_ps = psum_out_pool.tile([NCLS, B], f32, name="out_ps")

            def finish_tensor(ti, vals, start):
                """Copy the (identical-row) sum block to SBUF, pull row 0
                onto partitions per 128-subtile via a one-hot matmul,
                scale the SMALL fwt tiles by it (fwt[d,c]*S[d]), and
                accumulate (fwt*S).T @ trig into out_ps."""
                rows_sb = const_pool.tile([P, DW], f32, name=f"rows_sb{ti}")
                nc.vector.tensor_copy(rows_sb[:], rows[ti][:])
                for sub in range(NSUB):
                    ps = psum_pool.tile([P, 1], f32, name=f"ps{ti}_{sub}", tag="ps")
                    nc.tensor.matmul(
                        ps[:],
                        rows_sb[:, sub * P : (sub + 1) * P],
                        e0[:],
                        start=True,
                        stop=True,
                    )
                    fws = x_pool.tile(
                        [P, NCLS], bf16, name=f"fws{ti}_{sub}", tag=f"fws{ti}{sub}"
                    )
                    nc.vector.tensor_scalar_mul(fws[:], fwt[:, sub, :], ps[:])
                    nc.tensor.matmul(
                        out_ps[:],
                        fws[:],
                        vals[sub][:],
                        start=(start and sub == 0),
                        stop=(not start and sub == NSUB - 1),
                    )

            # a finishes mid-stream: its cos-side output matmuls overlap
            # the b stream (trig is ready ~10us in, well before rows_a).
            finish_tensor(0, coss, start=True)
            for gi, n in enumerate(GROUPS_B):
                load_group(1, gi, n)
            finish_tensor(1, sins, start=False)

            # Split store: two half-copies on different engines, two DMA
            # queues, so copy/DMA of the halves overlap.
            out_sb = const_pool.tile([NCLS, B], f32, name="out_sb")
            H = B // 2
            nc.scalar.copy(out_sb[:, 0:H], out_ps[:, 0:H])
            nc.scalar.dma_start(out=out_cb[:, 0:H], in_=out_sb[:, 0:H])
            nc.vector.tensor_copy(out_sb[:, H:B], out_ps[:, H:B])
            nc.sync.dma_start(out=out_cb[:, H:B], in_=out_sb[:, H:B])

    nc.compile()
    return nc


def _get_nc():
    if "nc" not in _STATE:
        _STATE["nc"] = _build()
    return _STATE["nc"]


def _prep_in_maps(x, a, b, fc_w):
    import ml_dtypes

    bf16 = ml_dtypes.bfloat16
    xf = np.asarray(x, dtype=np.float32).reshape(B, D)
    xtb = np.ascontiguousarray(xf.T).astype(bf16)  # [D, B] bf16
    a2 = np.asarray(a, dtype=np.float32).reshape(D, D).astype(bf16)
    b2 = np.asarray(b, dtype=np.float32).reshape(D, D).astype(bf16)
    fw = np.asarray(fc_w, dtype=np.float32)
    in_maps = []
    for m in range(NCORES):
        sl = slice(m * DW, (m + 1) * DW)
        im = {}
        for ti, (t2, sizes) in enumerate(
            ((a2, GROUPS_A), (b2, GROUPS_B))
        ):
            ts = t2[:, sl]
            c0 = 0
            for gi, n in enumerate(sizes):
                blk = (
                    ts[c0 * P : (c0 + n) * P, :]
                    .reshape(n, P, DW)
                    .transpose(1, 0, 2)
                )
                im[f"{'ab'[ti]}{gi}"] = np.ascontiguousarray(blk).reshape(
                    P, n * DW
                )
                c0 += n
        xs = xtb[sl, :].reshape(NSUB, P, B).transpose(1, 0, 2)
        im["xt_s"] = np.ascontiguousarray(xs).reshape(P, NSUB * B)
        fs = np.ascontiguousarray(fw[:, sl].T).reshape(NSUB, P, NCLS)
        im["fwt_s"] = np.ascontiguousarray(fs.transpose(1, 0, 2)).reshape(
            P, NSUB * NCLS
        )
        in_maps.append(im)
    return in_maps


def _run(inputs, trace=False, trace_kwargs=None):
    """Run the device kernel; returns (final_output, BassKernelResults)."""
    from concourse.bass_utils import run_bass_kernel_spmd

    x = inputs["x"]
    a = inputs["a"]
    b = inputs["b"]
    w = np.asarray(inputs["w"], dtype=np.float64)
    n_param = np.asarray(inputs["n_param"], dtype=np.float64)
    fc_w = np.asarray(inputs["fc_w"], dtype=np.float32)
    fc_b = np.asarray(inputs["fc_b"], dtype=np.float32)

    nc = _get_nc()
    in_maps = _prep_in_maps(x, a, b, fc_w)
    res = run_bass_kernel_spmd(
        nc,
        in_maps,
        list(range(NCORES)),
        trace=trace,
        **(trace_kwargs or {}),
    )

    acc = np.zeros((NCLS, B), dtype=np.float32)
    for r in res.results:
        acc += r["out_cb"]
    part1 = float(np.sum(w[1:] * n_param[1:] + w[:-1] * n_param[:-1]))
    final = acc.T + np.float32(part1) * fc_w.sum(axis=1)[None, :] + fc_b[None, :]
    return np.ascontiguousarray(final.astype(np.float32)), res


def kernel(**inputs) -> np.ndarray:
    out, _ = _run(inputs, trace=False)
    return out


# revision 6
# speedup vs baseline: 1.2865x; 1.0346x over previous
"""Trainium2 Bass kernel for nn_CustomCIFAR10Model.

Math (reference):
    xf = x.reshape(B, D)
    part2[b,d] = cos(xf[b,d]) * Sa[d] + sin(xf[b,d]) * Sb[d]
        where Sa[d] = sum_i a[i,d,0], Sb[d] = sum_i b[i,d,0]
    part1 = sum(w[1:]*n[1:] + w[:-1]*n[:-1])            (scalar)
    out = (part1 + part2) @ fc_w.T + fc_b               [B, NCLS]

The heavy part is reading a and b once to column-sum them: memory-bound.
Sharding: columns (d) split across 8 cores, 384 each. Every core
column-sums its a/b slice (PE matmuls), builds z = cos*Sa + sin*Sb for
its d-slice, and contracts against its fc_w columns, yielding a partial
[NCLS, B] output. Host sums the 8 partials and adds part1/bias.

a/b/x are cast to bf16 on the host (tolerance is 2e-2; measured error
~3e-3): halves the dominant HBM traffic vs f32 and doubles the PE
matmul rate. The column-sum matmuls use an all-ones [128,128] bf16
stationary so the [128, DW] output lands on all 128 PSUM partitions
(identical rows): a [1, DW] output would serialize on a single PSUM
write port, capping the stream at ~380ns/chunk instead of ~160ns.
Sa is then moved onto partitions with a one-hot e0 matmul in bf16
(stationary = SBUF copy of the sum rows, moving = e0 picks row 0).

Each group DMA reads one fully contiguous DRAM block (its own dram
tensor, packed partition-major on the host): a strided source costs
~20% of per-engine DMA bandwidth. All input DMAs ride one HWDGE queue
(sync) FIFO: a0 (2 chunks, so the PE starts ~5us earlier) -> fwt ->
xt -> rest of a -> b, 10 DMAs vs 8 HWDGE completion semaphores (the
two recycles wait on long-completed transfers). The last b group is
2 chunks so almost no matmul work remains after the final byte lands.

The PE HAM clock gate starts every kernel at 1.2 GHz and only ramps to
2.4 GHz after ~3.4us of sustained activity: a run of dummy matmuls on
a memset tile is issued first, burning the ramp during the preamble /
first-DMA dead time so every real matmul runs warm.

The output accumulates in one PSUM bank but is matmul'd per 256-col
batch half: the first half's store overlaps the second half's matmuls,
and partials store as bf16 (host upcasts and sums).

HW Sin only accepts [-pi, pi]: range-reduce t = x/(2pi), r = t - round(t)
via the fp32 magic-number trick, then Sin(2pi*r); cos shifts t by +1/4.
"""

import numpy as np

B = 512
D = 3072
NCLS = 100
P = 128
NCORES = 8
DW = D // NCORES          # 384 columns per core
NSUB = DW // P            # 3 d-subtiles of 128
NCH = D // P              # 24 row-chunks of a/b slice
GROUPS_A = [2, 6, 8, 8]   # small head group: PE starts early
GROUPS_B = [8, 8, 6, 2]   # small tail group: PE finishes early
NWARM = 11                # dummy matmuls to ramp the PE clock
H = B // 2

_STATE = {}


def _build():
    """Build + bacc-compile the SPMD Bass program (once per process)."""
    import concourse.bacc as bacc
    import concourse.mybir as mybir
    import concourse.tile as tile

    f32 = mybir.dt.float32
    bf16 = mybir.dt.bfloat16
    nc = bacc.Bacc(
        "TRN2", target_bir_lowering=False, debug=False, num_devices=NCORES
    )

    grp_srcs = [[], []]
    for ti, sizes in enumerate((GROUPS_A, GROUPS_B)):
        for gi, n in enumerate(sizes):
            grp_srcs[ti].append(
                nc.dram_tensor(
                    f"{'ab'[ti]}{gi}", [P, n * DW], bf16, kind="ExternalInput"
                )
            )
    xt_s = nc.dram_tensor("xt_s", [P, NSUB * B], bf16, kind="ExternalInput")
    fwt_s = nc.dram_tensor("fwt_s", [P, NSUB * NCLS], f32, kind="ExternalInput")
    out_cb = nc.dram_tensor("out_cb", [NCLS, B], bf16, kind="ExternalOutput")

    INV2PI = float(1.0 / (2.0 * np.pi))
    TWO_PI = float(2.0 * np.pi)
    MAGIC = float(1.5 * 2.0**23)
    add_op = mybir.AluOpType.add
    sub_op = mybir.AluOpType.subtract
    Sin = mybir.ActivationFunctionType.Sin
    Copy = mybir.ActivationFunctionType.Copy

    with tile.TileContext(nc) as tc:
        with (
            tc.tile_pool(name="chunks", bufs=5) as chunk_pool,
            tc.tile_pool(name="consts", bufs=1) as const_pool,
            tc.tile_pool(name="xwork", bufs=1) as x_pool,
            tc.tile_pool(name="ps", bufs=2, space="PSUM") as psum_pool,
            tc.tile_pool(name="psrow", bufs=1, space="PSUM") as psum_row_pool,
            tc.tile_pool(name="psout", bufs=1, space="PSUM") as psum_out_pool,
            tc.tile_pool(name="pswarm", bufs=1, space="PSUM") as psum_warm_pool,
        ):
            ones128 = const_pool.tile([P, P], bf16, name="ones128")
            nc.vector.memset(ones128[:], 1.0)
            e0 = const_pool.tile([P, 1], bf16, name="e0")
            nc.vector.memset(e0[:], 0.0)
            nc.vector.memset(e0[0:1, 0:1], 1.0)
            zero = const_pool.tile([P, 1], f32, name="zerob")
            nc.vector.memset(zero[:], 0.0)
            # Dummy Sin so the Sin table set loads once at kernel start.
            warm = const_pool.tile([P, 1], f32, name="warm")
            nc.scalar.activation(warm[:], zero[:], Sin, bias=zero[:])

            # PE clock ramp: ~3.4us of dummy matmuls while the PE would
            # otherwise idle waiting for the first chunk DMA.
            wsrc = const_pool.tile([P, B], bf16, name="wsrc")
            nc.vector.memset(wsrc[:], 0.0)
            wps = psum_warm_pool.tile([P, B], f32, name="wps")
            for _ in range(NWARM):
                nc.tensor.matmul(
                    wps[:], ones128[:], wsrc[:], start=True, stop=True
                )

            rows = []
            for ti in range(2):
                psr = psum_row_pool.tile(
                    [P, DW], f32, name=f"psr{ti}", tag=f"psr{ti}"
                )
                rows.append(psr)
            emitted = [0, 0]

            def load_group(ti, gi, n):
                """One DMA for one contiguous group + its matmuls."""
                ch = chunk_pool.tile(
                    [P, n, DW], bf16, name=f"ch{ti}_{gi}", tag="chunk"
                )
                nc.sync.dma_start(out=ch[:], in_=grp_srcs[ti][gi][:])
                for j in range(n):
                    nc.tensor.matmul(
                        rows[ti][:],
                        ones128[:],
                        ch[:, j, :],
                        start=(emitted[ti] == 0),
                        stop=(emitted[ti] == NCH - 1),
                    )
                    emitted[ti] += 1

            # First tiny a group ahead of everything: the PE's first real
            # matmul fires ~5us earlier than with a full 8-chunk group.
            load_group(0, 0, GROUPS_A[0])
            fwt = x_pool.tile([P, NSUB, NCLS], f32, name="fwt")
            nc.sync.dma_start(out=fwt[:], in_=fwt_s[:])
            xt = x_pool.tile([P, NSUB, B], bf16, name="xt")
            nc.sync.dma_start(out=xt[:], in_=xt_s[:])
            for gi, n in enumerate(GROUPS_A):
                if gi:
                    load_group(0, gi, n)

            # Trig on x while a/b stream: r = t - round(t) (magic trick),
            # then Sin(2pi*r); cos shifts t by +1/4 before rounding.
            # Sin writes bf16 directly (matmul moving operand, no cast op).
            sins = []
            coss = []
            for sub in range(NSUB):
                xts = xt[:, sub, :]
                ts_t = x_pool.tile([P, B], f32, name=f"ts{sub}", tag=f"ts{sub}")
                nc.scalar.activation(ts_t[:], xts, Copy, bias=0.0, scale=INV2PI)
                ks_t = x_pool.tile([P, B], f32, name=f"ks{sub}", tag=f"ks{sub}")
                nc.vector.tensor_scalar(ks_t[:], ts_t[:], MAGIC, MAGIC, add_op, sub_op)
                nc.vector.tensor_sub(ts_t[:], ts_t[:], ks_t[:])
                sinv = x_pool.tile([P, B], bf16, name=f"sin{sub}", tag=f"sin{sub}")
                nc.scalar.activation(
                    sinv[:], ts_t[:], Sin, bias=zero[:], scale=TWO_PI
                )
                sins.append(sinv)
                tc_t = x_pool.tile([P, B], f32, name=f"tc{sub}", tag=f"tc{sub}")
                nc.scalar.activation(tc_t[:], xts, Copy, bias=0.25, scale=INV2PI)
                kc_t = x_pool.tile([P, B], f32, name=f"kc{sub}", tag=f"kc{sub}")
                nc.vector.tensor_scalar(kc_t[:], tc_t[:], MAGIC, MAGIC, add_op, sub_op)
                nc.vector.tensor_sub(tc_t[:], tc_t[:], kc_t[:])
                cosv = x_pool.tile([P, B], bf16, name=f"cos{sub}", tag=f"cos{sub}")
                nc.scalar.activation(
                    cosv[:], tc_t[:], Sin, bias=zero[:], scale=TWO_PI
                )
                coss.append(cosv)

            out_ps = [
                psum_out_pool.tile([NCLS, H], f32, name=f"out_ps{h}", tag=f"out{h}")
                for h in range(2)
            ]

            def finish_tensor(ti, vals, start):
                """Copy the (identical-row) sum block to SBUF as bf16,
                pull row 0 onto partitions per 128-subtile via a one-hot
                matmul, scale the SMALL fwt tiles by it (fwt[d,c]*S[d]),
                and accumulate (fwt*S).T @ trig into out_ps, per batch
                half so the first half's store can overlap the second."""
                rows_sb = const_pool.tile([P, DW], bf16, name=f"rows_sb{ti}")
                nc.vector.tensor_copy(rows_sb[:], rows[ti][:])
                fwss = []
                for sub in range(NSUB):
                    ps = psum_pool.tile([P, 1], f32, name=f"ps{ti}_{sub}", tag="ps")
                    nc.tensor.matmul(
                        ps[:],
                        rows_sb[:, sub * P : (sub + 1) * P],
                        e0[:],
                        start=True,
                        stop=True,
                    )
                    fws = x_pool.tile(
                        [P, NCLS], bf16, name=f"fws{ti}_{sub}", tag=f"fws{ti}{sub}"
                    )
                    nc.vector.tensor_scalar_mul(fws[:], fwt[:, sub, :], ps[:])
                    fwss.append(fws)
                for half in range(2):
                    for sub in range(NSUB):
                        nc.tensor.matmul(
                            out_ps[half][:],
                            fwss[sub][:],
                            vals[sub][:, half * H : (half + 1) * H],
                            start=(start and sub == 0),
                            stop=(not start and sub == NSUB - 1),
                        )

            # a finishes mid-stream: its cos-side output matmuls overlap
            # the b stream (trig is ready ~13us in, well before rows_a).
            finish_tensor(0, coss, start=True)
            for gi, n in enumerate(GROUPS_B):
                load_group(1, gi, n)
            finish_tensor(1, sins, start=False)

            # Store per batch half: half 0's copy+DMA runs while half 1's
            # sin matmuls still execute; bf16 halves the bytes.
            out_sb = const_pool.tile([NCLS, B], bf16, name="out_sb")
            nc.scalar.copy(out_sb[:, 0:H], out_ps[0][:])
            nc.scalar.dma_start(out=out_cb[:, 0:H], in_=out_sb[:, 0:H])
            nc.vector.tensor_copy(out_sb[:, H:B], out_ps[1][:])
            nc.sync.dma_start(out=out_cb[:, H:B], in_=out_sb[:, H:B])

    nc.compile()
    return nc


def _get_nc():
    if "nc" not in _STATE:
        _STATE["nc"] = _build()
    return _STATE["nc"]


def _prep_in_maps(x, a, b, fc_w):
    import ml_dtypes

    bf16 = ml_dtypes.bfloat16
    xf = np.asarray(x, dtype=np.float32).reshape(B, D)
    xtb = np.ascontiguousarray(xf.T).astype(bf16)  # [D, B] bf16
    a2 = np.asarray(a, dtype=np.float32).reshape(D, D).astype(bf16)
    b2 = np.asarray(b, dtype=np.float32).reshape(D, D).astype(bf16)
    fw = np.asarray(fc_w, dtype=np.float32)
    in_maps = []
    for m in range(NCORES):
        sl = slice(m * DW, (m + 1) * DW)
        im = {}
        for ti, (t2, sizes) in enumerate(
            ((a2, GROUPS_A), (b2, GROUPS_B))
        ):
            ts = t2[:, sl]
            c0 = 0
            for gi, n in enumerate(sizes):
                blk = (
                    ts[c0 * P : (c0 + n) * P, :]
                    .reshape(n, P, DW)
                    .transpose(1, 0, 2)
                )
                im[f"{'ab'[ti]}{gi}"] = np.ascontiguousarray(blk).reshape(
                    P, n * DW
                )
                c0 += n
        xs = xtb[sl, :].reshape(NSUB, P, B).transpose(1, 0, 2)
        im["xt_s"] = np.ascontiguousarray(xs).reshape(P, NSUB * B)
        fs = np.ascontiguousarray(fw[:, sl].T).reshape(NSUB, P, NCLS)
        im["fwt_s"] = np.ascontiguousarray(fs.transpose(1, 0, 2)).reshape(
            P, NSUB * NCLS
        )
        in_maps.append(im)
    return in_maps


def _run(inputs, trace=False, trace_kwargs=None):
    """Run the device kernel; returns (final_output, BassKernelResults)."""
    from concourse.bass_utils import run_bass_kernel_spmd

    x = inputs["x"]
    a = inputs["a"]
    b = inputs["b"]
    w = np.asarray(inputs["w"], dtype=np.float64)
    n_param = np.asarray(inputs["n_param"], dtype=np.float64)
    fc_w = np.asarray(inputs["fc_w"], dtype=np.float32)
    fc_b = np.asarray(inputs["fc_b"], dtype=np.float32)

    nc = _get_nc()
    in_maps = _prep_in_maps(x, a, b, fc_w)
    res = run_bass_kernel_spmd(
        nc,
        in_maps,
        list(range(NCORES)),
        trace=trace,
        **(trace_kwargs or {}),
    )

    acc = np.zeros((NCLS, B), dtype=np.float32)
    for r in res.results:
        acc += np.asarray(r["out_cb"], dtype=np.float32)
    part1 = float(np.sum(w[1:] * n_param[1:] + w[:-1] * n_param[:-1]))
    final = acc.T + np.float32(part1) * fc_w.sum(axis=1)[None, :] + fc_b[None, :]
    return np.ascontiguousarray(final.astype(np.float32)), res


def kernel(**inputs) -> np.ndarray:
    out, _ = _run(inputs, trace=False)
    return out
